# revision 29
# baseline (speedup 1.0000x reference)
"""Trainium2 Bass kernel for nn_BERTRec_sequential (8-core batch-parallel).

Strategy:
- Data-parallel over batch: core c handles sequences [4c, 4c+4) end-to-end,
  no cross-core communication. Host concatenates per-core logits.
- Activations kept TRANSPOSED on-chip (h.T: [D partitions, tokens free]) so
  every projection is a natural TensorE matmul (lhsT = host-pre-transposed
  weights); LayerNorm partition-reductions via all-ones matmuls that yield
  partition-replicated stats for free.
- bf16 matmuls (fp32 PSUM accumulate), fp32 vector math. Host pre-casts
  weights/tables to bf16 and pre-transposes; embedding gathered on-device by
  indirect DMA then transposed via TensorE.
- Final [256 x 30001] projection: tokens on partitions so the 96MB/core logits
  write is contiguous 2KB DMA lines; out_b added via a K=1 ones-matmul psum.
"""

import math
import sys

sys.path.insert(0, "/opt/trn_rl_repo")

import numpy as np
import ml_dtypes

import concourse.bass as bass
import concourse.mybir as mybir
import concourse.tile as tile
from concourse import bacc
from concourse.bass_utils import run_bass_kernel_spmd

BF16 = ml_dtypes.bfloat16
F32 = np.float32

B, L = 32, 200
ITEM = 30000
V = ITEM + 2
D, H, NL, FFD = 256, 4, 2, 1024
DK = D // H
EMB = [768, 128, 512, 128]
LHS = [512, 320, 384, 224]
NCORE = 8
SPC = B // NCORE              # sequences per core = 4
TOK = SPC * L                 # 800 tokens per core
NTCH = (TOK + 127) // 128     # 7 token chunks
TOKP = NTCH * 128             # 896
VOUT = ITEM + 1               # 30001
VT = 512                      # vocab tile for final projection
FH = TOK // 2                 # 400, free-dim half for PSUM-sized matmuls
NEG = -1.0e9


def _chunks(n, step=128):
    out = []
    i = 0
    while i < n:
        out.append((i, min(step, n - i)))
        i += step
    return out


def _attn_chunks():
    """Per-sequence key chunks: [128, 72] starting at the sequence base, so
    every chunk lands at partition base 0 in the per-sequence V tiles
    (TensorE requires operand base partition in {0, 32, 64})."""
    return [[(s * L, 128), (s * L + 128, L - 128)] for s in range(SPC)]


ATTN = _attn_chunks()
NMASK = sum(len(c) for c in ATTN)  # 8
MASK_COL = {}
_c = 0
for _s in range(SPC):
    for _ci in range(len(ATTN[_s])):
        MASK_COL[(_s, _ci)] = _c
        _c += 1

VTILES = _chunks(VOUT, VT)    # 58x512 + 305
TCH = _chunks(TOK, 128)       # token chunks: 6x128 + 32


def _build_program(zero_bias):
    nc = bacc.Bacc(None, target_bir_lowering=False, debug=False)
    dt = mybir.dt
    AF = mybir.ActivationFunctionType
    OP = mybir.AluOpType

    from contextlib import ExitStack
    with tile.TileContext(nc) as tc, ExitStack() as ctx:
        dram = ctx.enter_context(tc.tile_pool(name="dram", bufs=1, space="DRAM"))

        def din(name, shape, dtype):
            return dram.tile(shape, dtype, kind="ExternalInput", name=name,
                             uniquify=False)

        idx_d = din("idx", [128, NTCH], dt.int32)
        mask_d = din("mask", [128, NMASK], dt.float32)
        twd = []
        for t in range(4):
            emb, lhs = EMB[t], LHS[t]
            nch1 = len(_chunks(lhs))
            d_ = {
                "table": din(f"t{t}_table", [V, emb], dt.bfloat16),
                "w1T": din(f"t{t}_w1T", [emb, lhs], dt.bfloat16),
                "b1": din(f"t{t}_b1", [128, nch1], dt.float32),
                "w2T": din(f"t{t}_w2T", [nch1 * 128, D], dt.bfloat16),
                "b2": din(f"t{t}_b2", [128, 2], dt.float32),
                "posT": din(f"t{t}_posT", [D, L], dt.float32),
                "ln0": din(f"t{t}_ln0", [128, 4], dt.float32),
                "top": din(f"t{t}_top", [128, 4], dt.float32),
                "layers": [],
            }
            for l in range(NL):
                p = f"t{t}_l{l}_"
                d_["layers"].append({
                    "wqT": din(p + "wqT", [D, D], dt.bfloat16),
                    "wkT": din(p + "wkT", [D, D], dt.bfloat16),
                    "wvT": din(p + "wvT", [D, D], dt.bfloat16),
                    "woT": din(p + "woT", [D, D], dt.bfloat16),
                    "bq": din(p + "bq", [128, 2], dt.float32),
                    "bk": din(p + "bk", [128, 2], dt.float32),
                    "bv": din(p + "bv", [D], dt.float32),
                    "bo": din(p + "bo", [128, 2], dt.float32),
                    "ln1": din(p + "ln1", [128, 4], dt.float32),
                    "ln2": din(p + "ln2", [128, 4], dt.float32),
                    "fw1T": din(p + "fw1T", [D, FFD], dt.bfloat16),
                    "fb1": din(p + "fb1", [128, FFD // 128], dt.float32),
                    "fw2T": din(p + "fw2T", [FFD, D], dt.bfloat16),
                    "fb2": din(p + "fb2", [128, 2], dt.float32),
                })
            twd.append(d_)
        outw_d = din("out_wT", [D, VOUT], dt.bfloat16)
        outb_d = din("out_b", [1, VOUT], dt.bfloat16)
        out_d = dram.tile([TOK, VOUT], dt.float32, kind="ExternalOutput",
                          name="out", uniquify=False)

        const = ctx.enter_context(tc.tile_pool(name="const", bufs=1))
        work = ctx.enter_context(tc.tile_pool(name="work", bufs=2))
        wpool = ctx.enter_context(tc.tile_pool(name="wpool", bufs=2))
        espool = ctx.enter_context(tc.tile_pool(name="espool", bufs=8))
        lpool = ctx.enter_context(tc.tile_pool(name="lpool", bufs=3))
        ps = ctx.enter_context(tc.tile_pool(name="ps", bufs=4, space="PSUM"))

        _PS_BUFS = {"ps": 4, "ps_sum": 2, "ps_rep": 1, "ps_av": 1}

        def psum(p_, f_, tag="ps"):
            return ps.tile([p_, f_], dt.float32, space="PSUM", tag=tag,
                           name=tag, bufs=_PS_BUFS[tag])

        # constants
        ones_stat = const.tile([128, 128], dt.bfloat16)   # value 2^-8
        nc.vector.memset(ones_stat, 2.0 ** -8)
        eps_std = const.tile([128, 1], dt.float32)
        nc.vector.memset(eps_std, 1e-5)
        eps_cus = const.tile([128, 1], dt.float32)
        nc.vector.memset(eps_cus, 1e-6)
        ones_col = const.tile([128, 1], dt.bfloat16)
        nc.vector.memset(ones_col, 1.0)
        ones_row = const.tile([1, 128], dt.bfloat16)
        nc.vector.memset(ones_row, 1.0)
        ident = const.tile([128, 128], dt.bfloat16)
        from concourse.masks import make_identity
        make_identity(nc, ident)
        idx_sb = const.tile([128, NTCH], dt.int32)
        nc.sync.dma_start(idx_sb, idx_d[:])
        mask_sb = const.tile([128, NMASK], dt.float32)
        nc.sync.dma_start(mask_sb, mask_d[:])

        yT = const.tile([128, 2, TOK], dt.float32)   # tower-sum accumulator
        nc.vector.memset(yT[:, 0, :], 0.0)
        nc.vector.memset(yT[:, 1, :], 0.0)

        def layernorm(r, kind, wb_sb, xn):
            """r: [128,2,TOK] bf16 pre-norm. wb_sb: [128,4] (w0,w1,b0,b1).
            Writes xn (bf16 or fp32, per tile dtype)."""
            sq = work.tile([128, 2, TOK], dt.bfloat16, tag="ln_sq", bufs=1)
            for k in range(2):
                nc.vector.tensor_mul(sq[:, k, :], r[:, k, :], r[:, k, :])
            for f in range(2):
                fs = slice(f * FH, (f + 1) * FH)
                pm = psum(128, FH)
                pq = psum(128, FH)
                for k in range(2):
                    nc.tensor.matmul(pm[:], lhsT=ones_stat[:], rhs=r[:, k, fs],
                                     start=(k == 0), stop=(k == 1))
                for k in range(2):
                    nc.tensor.matmul(pq[:], lhsT=ones_stat[:], rhs=sq[:, k, fs],
                                     start=(k == 0), stop=(k == 1))
                # pm = mean (replicated), pq = mean-square (replicated)
                mean_sb = work.tile([128, FH], dt.float32, tag="ln_mean")
                nc.scalar.activation(mean_sb[:], pm[:], AF.Copy)
                var = work.tile([128, FH], dt.float32, tag="ln_var")
                m2 = work.tile([128, FH], dt.float32, tag="ln_m2")
                nc.vector.tensor_mul(m2[:], mean_sb[:], mean_sb[:])
                nc.vector.tensor_sub(var[:], pq[:], m2[:])
                if kind == "std":
                    # rstd = 1/sqrt(var + 1e-5)
                    nc.scalar.activation(var[:], var[:], AF.Abs_reciprocal_sqrt,
                                         bias=eps_std[:])
                else:
                    # 1/(sqrt(var*256/255) + 1e-6) ~= ars(var*256/255)
                    # (eps/s <= 1e-5 relative, far below bf16 noise)
                    nc.scalar.activation(var[:], var[:], AF.Abs_reciprocal_sqrt,
                                         scale=256.0 / 255.0)
                for k in range(2):
                    xs = xn[:, k, fs]
                    nc.vector.tensor_sub(xs, r[:, k, fs], mean_sb[:])
                    nc.vector.tensor_mul(xs, xs, var[:])
                    nc.vector.tensor_scalar(
                        out=xs, in0=xs,
                        scalar1=wb_sb[:, k:k + 1], scalar2=wb_sb[:, 2 + k:3 + k],
                        op0=OP.mult, op1=OP.add)

        for t in range(4):
            emb, lhs = EMB[t], LHS[t]
            nesub = emb // 128
            lch = _chunks(lhs)
            td = twd[t]

            # ---- embedding gather + transpose -> ET [128, nesub, TOKP] bf16
            ET = work.tile([128, nesub, TOKP], dt.bfloat16, tag="ET", bufs=2)
            for k in range(NTCH):
                eg = work.tile([128, emb], dt.bfloat16, tag="eg", bufs=3)
                nc.gpsimd.indirect_dma_start(
                    out=eg[:], out_offset=None, in_=td["table"][:],
                    in_offset=bass.IndirectOffsetOnAxis(
                        ap=idx_sb[:, k:k + 1], axis=0))
                for s in range(nesub):
                    pt = ps.tile([128, 128], dt.bfloat16, space="PSUM",
                                 tag="ps", name="pt", bufs=4)
                    nc.tensor.transpose(pt[:], eg[:, 128 * s:128 * (s + 1)],
                                        ident[:])
                    nc.vector.tensor_copy(ET[:, s, 128 * k:128 * (k + 1)], pt[:])

            # ---- MLP1: A1T = lrelu(w1 @ E.T + b1)
            w1_sb = wpool.tile([128, nesub, lhs], dt.bfloat16, tag="w1", bufs=1)
            nc.sync.dma_start(w1_sb, td["w1T"][:].rearrange(
                "(e p) l -> p e l", p=128))
            b1_sb = wpool.tile([128, len(lch)], dt.float32, tag="b1")
            nc.sync.dma_start(b1_sb, td["b1"][:])
            A1T = work.tile([128, len(lch), TOK], dt.bfloat16, tag="A1T", bufs=1)
            for c, (ms, msz) in enumerate(lch):
                pf = [psum(128, FH), psum(128, FH)]
                for s in range(nesub):
                    for f in range(2):
                        nc.tensor.matmul(pf[f][:msz],
                                         lhsT=w1_sb[:, s, ms:ms + msz],
                                         rhs=ET[:, s, f * FH:(f + 1) * FH],
                                         start=(s == 0), stop=(s == nesub - 1))
                for f in range(2):
                    nc.scalar.activation(A1T[:msz, c, f * FH:(f + 1) * FH],
                                         pf[f][:msz], AF.Lrelu,
                                         bias=b1_sb[:msz, c:c + 1], alpha=0.01)

            # ---- MLP2: h = lrelu(w2 @ A1.T + b2) -> r fp32 [128,2,TOK]
            w2_sb = wpool.tile([128, len(lch), D], dt.bfloat16, tag="w2")
            nc.sync.dma_start(w2_sb, td["w2T"][:].rearrange(
                "(c p) d -> p c d", p=128))
            b2_sb = wpool.tile([128, 2], dt.float32, tag="b2")
            nc.sync.dma_start(b2_sb, td["b2"][:])
            pos_sb = wpool.tile([128, 2, L], dt.float32, tag="pos")
            nc.sync.dma_start(pos_sb, td["posT"][:].rearrange(
                "(k p) t -> p k t", p=128))
            r = work.tile([128, 2, TOK], dt.bfloat16, tag="resid")
            for m in range(2):
                pf = [psum(128, FH), psum(128, FH)]
                for c, (ks, ksz) in enumerate(lch):
                    for f in range(2):
                        nc.tensor.matmul(pf[f][:],
                                         lhsT=w2_sb[:ksz, c,
                                                    128 * m:128 * (m + 1)],
                                         rhs=A1T[:ksz, c, f * FH:(f + 1) * FH],
                                         start=(c == 0), stop=(c == len(lch) - 1))
                for f in range(2):
                    nc.scalar.activation(r[:, m, f * FH:(f + 1) * FH],
                                         pf[f][:], AF.Lrelu,
                                         bias=b2_sb[:, m:m + 1], alpha=0.01)
            # + positional embedding (per sequence)
            for m in range(2):
                for s in range(SPC):
                    sl = slice(s * L, s * L + L)
                    nc.vector.tensor_add(r[:, m, sl], r[:, m, sl],
                                         pos_sb[:, m, :])

            # ---- ln0 (std)
            ln0_sb = wpool.tile([128, 4], dt.float32, tag="ln0")
            nc.sync.dma_start(ln0_sb, td["ln0"][:])
            xn = work.tile([128, 2, TOK], dt.bfloat16, tag="xn")
            layernorm(r, "std", ln0_sb, xn)

            # ---- transformer layers
            for l in range(NL):
                ld = td["layers"][l]
                wq_sb = wpool.tile([128, 2, D], dt.bfloat16, tag="wq")
                wk_sb = wpool.tile([128, 2, D], dt.bfloat16, tag="wk")
                wv_sb = wpool.tile([128, 2, D], dt.bfloat16, tag="wv")
                wo_sb = wpool.tile([128, 2, D], dt.bfloat16, tag="wo")
                for sb_, nm in ((wq_sb, "wqT"), (wk_sb, "wkT"),
                                (wv_sb, "wvT"), (wo_sb, "woT")):
                    nc.sync.dma_start(sb_, ld[nm][:].rearrange(
                        "(k p) d -> p k d", p=128))
                bq_sb = wpool.tile([128, 2], dt.float32, tag="bq")
                nc.sync.dma_start(bq_sb, ld["bq"][:])
                bk_sb = wpool.tile([128, 2], dt.float32, tag="bk")
                nc.sync.dma_start(bk_sb, ld["bk"][:])
                bo_sb = wpool.tile([128, 2], dt.float32, tag="bo")
                nc.sync.dma_start(bo_sb, ld["bo"][:])
                bv_rep = wpool.tile([128, D], dt.float32, tag="bv")
                nc.sync.dma_start(bv_rep, bass.AP(
                    tensor=ld["bv"].tensor, offset=ld["bv"].offset,
                    ap=[[0, 128]] + list(ld["bv"].ap)))

                # Q.T, K.T: [128, 2, TOK] bf16
                QT = work.tile([128, 2, TOK], dt.bfloat16, tag="QT")
                KT = work.tile([128, 2, TOK], dt.bfloat16, tag="KT")
                for dst, w_sb, bias_sb in ((QT, wq_sb, bq_sb), (KT, wk_sb, bk_sb)):
                    for m in range(2):
                        pf = [psum(128, FH), psum(128, FH)]
                        for k in range(2):
                            for f in range(2):
                                nc.tensor.matmul(
                                    pf[f][:],
                                    lhsT=w_sb[:, k, 128 * m:128 * (m + 1)],
                                    rhs=xn[:, k, f * FH:(f + 1) * FH],
                                    start=(k == 0), stop=(k == 1))
                        for f in range(2):
                            nc.vector.tensor_scalar_add(
                                dst[:, m, f * FH:(f + 1) * FH], pf[f][:],
                                bias_sb[:, m:m + 1])
                # V (token-major, per-sequence aligned): [128, 2*SPC, D] bf16
                Vb = work.tile([128, 2 * SPC, D], dt.bfloat16, tag="Vb", bufs=1)
                for s in range(SPC):
                    for c2, (js, jsz) in enumerate(ATTN[s]):
                        pp = psum(128, D)
                        for k in range(2):
                            nc.tensor.matmul(pp[:jsz],
                                             lhsT=xn[:, k, js:js + jsz],
                                             rhs=wv_sb[:, k, :],
                                             start=(k == 0), stop=(k == 1))
                        nc.vector.tensor_add(Vb[:jsz, 2 * s + c2, :], pp[:jsz],
                                             bv_rep[:jsz, :])

                # attention -> OT [128, 2, TOK] bf16
                OT = work.tile([128, 2, TOK], dt.bfloat16, tag="OT")
                for s in range(SPC):
                    sl = slice(s * L, s * L + L)
                    nch = len(ATTN[s])
                    es_all = {}
                sums_sb = espool.tile([1, SPC * H * L], dt.bfloat16,
                                      tag="sums_sb", bufs=1, name="sums_sb")
                for s in range(SPC):
                    sl = slice(s * L, s * L + L)
                    nch = len(ATTN[s])
                    pss = [psum(1, 2 * L, tag="ps_sum"),
                           psum(1, 2 * L, tag="ps_sum")]
                    for h in range(H):
                        k2, r0 = h // 2, 64 * (h % 2)
                        rr = slice(r0, r0 + 64)
                        sumsl = slice((h % 2) * L, (h % 2) * L + L)
                        for ci, (gs, sz) in enumerate(ATTN[s]):
                            pS = psum(128, L)
                            nc.tensor.matmul(pS[:sz],
                                             lhsT=KT[rr, k2, gs:gs + sz],
                                             rhs=QT[rr, k2, sl],
                                             start=True, stop=True)
                            es = espool.tile([128, L], dt.bfloat16, tag="es",
                                             bufs=34, name="es")
                            mc = MASK_COL[(s, ci)]
                            nc.scalar.activation(
                                es[:sz], pS[:sz], AF.Exp,
                                bias=mask_sb[:sz, mc:mc + 1],
                                scale=1.0 / math.sqrt(DK))
                            es_all[(s, h, ci)] = (es, sz)
                            nc.tensor.matmul(pss[h // 2][:, sumsl],
                                             lhsT=ones_col[:sz], rhs=es[:sz],
                                             start=(ci == 0),
                                             stop=(ci == nch - 1))
                    for g in range(2):
                        nc.scalar.activation(
                            sums_sb[:, s * H * L + 2 * g * L:
                                    s * H * L + 2 * (g + 1) * L],
                            pss[g][:], AF.Copy)
                # one ars + one square for the whole layer-tower
                nc.scalar.activation(sums_sb[:], sums_sb[:],
                                     AF.Abs_reciprocal_sqrt)
                nc.scalar.activation(sums_sb[:], sums_sb[:], AF.Square)
                for s in range(SPC):
                    sl = slice(s * L, s * L + L)
                    nch = len(ATTN[s])
                    rep_sb = espool.tile([128, H * L], dt.bfloat16,
                                         tag="rep_sb", bufs=2, name="rep_sb")
                    for g in range(2):
                        prep = psum(128, 2 * L, tag="ps_rep")
                        nc.tensor.matmul(
                            prep[:], lhsT=ones_row[:],
                            rhs=sums_sb[:, s * H * L + 2 * g * L:
                                        s * H * L + 2 * (g + 1) * L],
                            start=True, stop=True)
                        nc.scalar.activation(
                            rep_sb[:, 2 * g * L:2 * (g + 1) * L], prep[:],
                            AF.Copy)
                    for h in range(H):
                        k2, r0 = h // 2, 64 * (h % 2)
                        rr = slice(r0, r0 + 64)
                        pO = psum(64, L, tag="ps_av")
                        for ci in range(nch):
                            es, sz = es_all[(s, h, ci)]
                            nc.tensor.matmul(pO[:],
                                             lhsT=Vb[:sz, 2 * s + ci,
                                                     64 * h:64 * h + 64],
                                             rhs=es[:sz],
                                             start=(ci == 0),
                                             stop=(ci == nch - 1))
                        nc.vector.tensor_mul(OT[rr, k2, sl], pO[:],
                                             rep_sb[:64, h * L:(h + 1) * L])
                # out-proj + residual -> r_new
                r_new = work.tile([128, 2, TOK], dt.bfloat16, tag="resid")
                for m in range(2):
                    pf = [psum(128, FH), psum(128, FH)]
                    for k in range(2):
                        for f in range(2):
                            nc.tensor.matmul(pf[f][:],
                                             lhsT=wo_sb[:, k, 128 * m:128 * (m + 1)],
                                             rhs=OT[:, k, f * FH:(f + 1) * FH],
                                             start=(k == 0), stop=(k == 1))
                    for f in range(2):
                        fs = slice(f * FH, (f + 1) * FH)
                        nc.vector.scalar_tensor_tensor(
                            out=r_new[:, m, fs], in0=pf[f][:],
                            scalar=bo_sb[:, m:m + 1], in1=xn[:, m, fs],
                            op0=OP.add, op1=OP.add)
                ln1_sb = wpool.tile([128, 4], dt.float32, tag="ln1")
                nc.sync.dma_start(ln1_sb, ld["ln1"][:])
                xn = work.tile([128, 2, TOK], dt.bfloat16, tag="xn")
                layernorm(r_new, "custom", ln1_sb, xn)

                # FF
                fw1_sb = wpool.tile([128, 2, FFD], dt.bfloat16, tag="fw1", bufs=1)
                nc.sync.dma_start(fw1_sb, ld["fw1T"][:].rearrange(
                    "(k p) f -> p k f", p=128))
                fb1_sb = wpool.tile([128, FFD // 128], dt.float32, tag="fb1")
                nc.sync.dma_start(fb1_sb, ld["fb1"][:])
                fw2_sb = wpool.tile([128, FFD // 128, D], dt.bfloat16, tag="fw2", bufs=1)
                nc.sync.dma_start(fw2_sb, ld["fw2T"][:].rearrange(
                    "(c p) d -> p c d", p=128))
                fb2_sb = wpool.tile([128, 2], dt.float32, tag="fb2")
                nc.sync.dma_start(fb2_sb, ld["fb2"][:])

                G = work.tile([128, FFD // 128, TOK], dt.bfloat16, tag="G", bufs=1)
                for c in range(FFD // 128):
                    pf = [psum(128, FH), psum(128, FH)]
                    for k in range(2):
                        for f in range(2):
                            nc.tensor.matmul(pf[f][:],
                                             lhsT=fw1_sb[:, k, 128 * c:128 * (c + 1)],
                                             rhs=xn[:, k, f * FH:(f + 1) * FH],
                                             start=(k == 0), stop=(k == 1))
                    for f in range(2):
                        nc.scalar.activation(G[:, c, f * FH:(f + 1) * FH],
                                             pf[f][:], AF.Gelu_apprx_tanh,
                                             bias=fb1_sb[:, c:c + 1])
                r_new = work.tile([128, 2, TOK], dt.bfloat16, tag="resid")
                for m in range(2):
                    pf = [psum(128, FH), psum(128, FH)]
                    for c in range(FFD // 128):
                        for f in range(2):
                            nc.tensor.matmul(pf[f][:],
                                             lhsT=fw2_sb[:, c, 128 * m:128 * (m + 1)],
                                             rhs=G[:, c, f * FH:(f + 1) * FH],
                                             start=(c == 0),
                                             stop=(c == FFD // 128 - 1))
                    for f in range(2):
                        fs = slice(f * FH, (f + 1) * FH)
                        nc.vector.scalar_tensor_tensor(
                            out=r_new[:, m, fs], in0=pf[f][:],
                            scalar=fb2_sb[:, m:m + 1], in1=xn[:, m, fs],
                            op0=OP.add, op1=OP.add)
                ln2_sb = wpool.tile([128, 4], dt.float32, tag="ln2")
                nc.sync.dma_start(ln2_sb, ld["ln2"][:])
                xn = work.tile([128, 2, TOK], dt.bfloat16, tag="xn")
                layernorm(r_new, "custom", ln2_sb, xn)

            # ---- top std_ln, accumulate into yT
            top_sb = wpool.tile([128, 4], dt.float32, tag="top")
            nc.sync.dma_start(top_sb, td["top"][:])
            tn = work.tile([128, 2, TOK], dt.float32, tag="tn")
            layernorm(xn, "std", top_sb, tn)
            for k in range(2):
                nc.vector.tensor_add(yT[:, k, :], yT[:, k, :], tn[:, k, :])

        # ---- final projection: out[tok, vocab] = y @ out_w.T + out_b
        yTb = const.tile([128, 2, TOK], dt.bfloat16)
        for k in range(2):
            nc.vector.tensor_copy(yTb[:, k, :], yT[:, k, :])
        ROT = ["ps", "ps", "ps", "ps", "ps_sum", "ps_sum", "ps_rep",
               "ps_av"]
        rot_i = 0
        for ni, (ns, nsz) in enumerate(VTILES):
            w_sb = lpool.tile([128, 2, VT], dt.bfloat16, tag="wv", bufs=3,
                              name="wv")
            nc.sync.dma_start(
                w_sb[:, :, :nsz],
                outw_d[:, ns:ns + nsz].rearrange("(k p) v -> p k v", p=128))
            if not zero_bias:
                ob = lpool.tile([1, VT], dt.bfloat16, tag="ob", bufs=3,
                                name="ob")
                nc.sync.dma_start(ob[:, :nsz], outb_d[0:1, ns:ns + nsz])
                pb = psum(128, VT, tag="ps_rep")
                nc.tensor.matmul(pb[:, :nsz], lhsT=ones_row[:],
                                 rhs=ob[:, :nsz], start=True, stop=True)
                bias_sb = lpool.tile([128, VT], dt.float32, tag="pbias",
                                     bufs=2, name="pbias")
                nc.vector.tensor_copy(bias_sb[:, :nsz], pb[:, :nsz])
            lsb = lpool.tile([128, NTCH, VT], dt.float32, tag="lsb", bufs=2,
                             name="lsb")
            for mi, (ms, msz) in enumerate(TCH):
                pl = psum(128, VT, tag=ROT[rot_i % len(ROT)])
                rot_i += 1
                for k in range(2):
                    nc.tensor.matmul(pl[:msz, :nsz],
                                     lhsT=yTb[:, k, ms:ms + msz],
                                     rhs=w_sb[:, k, :nsz],
                                     start=(k == 0), stop=(k == 1))
                if zero_bias:
                    nc.vector.tensor_copy(lsb[:msz, mi, :nsz], pl[:msz, :nsz])
                else:
                    nc.vector.tensor_add(lsb[:msz, mi, :nsz], pl[:msz, :nsz],
                                         bias_sb[:msz, :nsz])
            nc.gpsimd.dma_start(
                out_d[:768, ns:ns + nsz].rearrange("(c p) v -> p c v", p=128),
                lsb[:, :6, :nsz])
            nc.gpsimd.dma_start(out_d[768:, ns:ns + nsz], lsb[:32, 6, :nsz])

    nc.compile()
    return nc


_NC = {}


def _get_nc(zero_bias):
    if zero_bias not in _NC:
        _NC[zero_bias] = _build_program(zero_bias)
    return _NC[zero_bias]


def _pack_pp(v):
    """[256] -> [128, 2] column-per-dsub layout."""
    return np.ascontiguousarray(v.reshape(2, 128).T.astype(F32))


def _pack_pp_n(v, n=128):
    k = (len(v) + n - 1) // n
    buf = np.zeros((k * n,), F32)
    buf[:len(v)] = v
    return np.ascontiguousarray(buf.reshape(k, n).T)


def _pack_ln(w, b):
    return np.ascontiguousarray(
        np.stack([w[:128], w[128:], b[:128], b[128:]], axis=1).astype(F32))


def _prep_weights(t1, t2, t3, t4, out_w, out_b):
    m = {}
    for t, td in enumerate((t1, t2, t3, t4)):
        lhs = LHS[t]
        nch1 = len(_chunks(lhs))
        m[f"t{t}_table"] = np.asarray(td["table"]).astype(BF16)
        m[f"t{t}_w1T"] = np.ascontiguousarray(
            np.asarray(td["w1"]).T).astype(BF16)
        m[f"t{t}_b1"] = _pack_pp_n(np.asarray(td["b1"]))
        w2T = np.zeros((nch1 * 128, D), F32)
        w2T[:lhs] = np.asarray(td["w2"]).T
        m[f"t{t}_w2T"] = w2T.astype(BF16)
        m[f"t{t}_b2"] = _pack_pp(np.asarray(td["b2"]))
        m[f"t{t}_posT"] = np.ascontiguousarray(
            np.asarray(td["pos"]).T.astype(F32))
        m[f"t{t}_ln0"] = _pack_ln(np.asarray(td["ln0_w"]),
                                  np.asarray(td["ln0_b"]))
        m[f"t{t}_top"] = _pack_ln(np.asarray(td["top_w"]),
                                  np.asarray(td["top_b"]))
        for l, ld in enumerate(td["layers"]):
            p = f"t{t}_l{l}_"
            for nm in ("wq", "wk", "wv", "wo"):
                m[p + nm + "T"] = np.ascontiguousarray(
                    np.asarray(ld[nm]).T).astype(BF16)
            m[p + "bq"] = _pack_pp(np.asarray(ld["bq"]))
            m[p + "bk"] = _pack_pp(np.asarray(ld["bk"]))
            m[p + "bv"] = np.asarray(ld["bv"]).astype(F32)
            m[p + "bo"] = _pack_pp(np.asarray(ld["bo"]))
            m[p + "ln1"] = _pack_ln(np.asarray(ld["a1"]), np.asarray(ld["c1"]))
            m[p + "ln2"] = _pack_ln(np.asarray(ld["a2"]), np.asarray(ld["c2"]))
            m[p + "fw1T"] = np.ascontiguousarray(
                np.asarray(ld["fw1"]).T).astype(BF16)
            m[p + "fb1"] = _pack_pp_n(np.asarray(ld["fb1"]))
            m[p + "fw2T"] = np.ascontiguousarray(
                np.asarray(ld["fw2"]).T).astype(BF16)
            m[p + "fb2"] = _pack_pp(np.asarray(ld["fb2"]))
    m["out_wT"] = np.ascontiguousarray(np.asarray(out_w).T).astype(BF16)
    m["out_b"] = np.asarray(out_b).reshape(1, VOUT).astype(BF16)
    return m


def _prep_core(x, c):
    """Per-core idx [128, NTCH] int32 and mask [128, NMASK] fp32."""
    xs = np.asarray(x[SPC * c:SPC * (c + 1)]).astype(np.int64)  # [4, 200]
    flat = xs.reshape(-1)  # [800]
    idx = np.zeros((NTCH * 128,), np.int32)
    idx[:TOK] = flat.astype(np.int32)
    idx = np.ascontiguousarray(idx.reshape(NTCH, 128).T)
    mask = np.full((128, NMASK), NEG, F32)
    for s in range(SPC):
        for ci, (gs, sz) in enumerate(ATTN[s]):
            col = MASK_COL[(s, ci)]
            valid = xs[s, gs - s * L: gs - s * L + sz] > 0
            mask[:sz, col] = np.where(valid, 0.0, NEG)
    return idx, mask


def _run(inputs, trace=False):
    x = np.asarray(inputs["x"])
    wm = _prep_weights(inputs["t1"], inputs["t2"], inputs["t3"], inputs["t4"],
                       inputs["out_w"], inputs["out_b"])
    zero_bias = not np.any(np.asarray(inputs["out_b"]))
    nc = _get_nc(zero_bias)
    in_maps = []
    for c in range(NCORE):
        idx, mask = _prep_core(x, c)
        m = dict(wm)
        m["idx"] = idx
        m["mask"] = mask
        in_maps.append(m)
    res = run_bass_kernel_spmd(nc, in_maps, core_ids=list(range(NCORE)),
                               trace=trace)
    outs = [res.results[c]["out"].reshape(SPC, L, VOUT) for c in range(NCORE)]
    full = np.concatenate(outs, axis=0).astype(F32)
    return full, res


def kernel(**inputs):
    out, _ = _run(inputs, trace=False)
    return out


# revision 30
# speedup vs baseline: 1.0561x; 1.0561x over previous
"""Trainium2 Bass kernel for nn_BERTRec_sequential (8-core batch-parallel).

Strategy:
- Data-parallel over batch: core c handles sequences [4c, 4c+4) end-to-end,
  no cross-core communication. Host concatenates per-core logits.
- Activations kept TRANSPOSED on-chip (h.T: [D partitions, tokens free]) so
  every projection is a natural TensorE matmul (lhsT = host-pre-transposed
  weights); LayerNorm partition-reductions via all-ones matmuls that yield
  partition-replicated stats for free.
- bf16 matmuls (fp32 PSUM accumulate), fp32 vector math. Host pre-casts
  weights/tables to bf16 and pre-transposes; embedding gathered on-device by
  indirect DMA then transposed via TensorE.
- Final [256 x 30001] projection: tokens on partitions so the 96MB/core logits
  write is contiguous 2KB DMA lines; out_b added via a K=1 ones-matmul psum.
"""

import math
import sys

sys.path.insert(0, "/opt/trn_rl_repo")

import numpy as np
import ml_dtypes

import concourse.bass as bass
import concourse.mybir as mybir
import concourse.tile as tile
from concourse import bacc
from concourse.bass_utils import run_bass_kernel_spmd

BF16 = ml_dtypes.bfloat16
F32 = np.float32

B, L = 32, 200
ITEM = 30000
V = ITEM + 2
D, H, NL, FFD = 256, 4, 2, 1024
DK = D // H
EMB = [768, 128, 512, 128]
LHS = [512, 320, 384, 224]
NCORE = 8
SPC = B // NCORE              # sequences per core = 4
TOK = SPC * L                 # 800 tokens per core
NTCH = (TOK + 127) // 128     # 7 token chunks
TOKP = NTCH * 128             # 896
VOUT = ITEM + 1               # 30001
VT = 512                      # vocab tile for final projection
FH = TOK // 2                 # 400, free-dim half for PSUM-sized matmuls
NEG = -1.0e9


def _chunks(n, step=128):
    out = []
    i = 0
    while i < n:
        out.append((i, min(step, n - i)))
        i += step
    return out


def _attn_chunks():
    """Per-sequence key chunks: [128, 72] starting at the sequence base, so
    every chunk lands at partition base 0 in the per-sequence V tiles
    (TensorE requires operand base partition in {0, 32, 64})."""
    return [[(s * L, 128), (s * L + 128, L - 128)] for s in range(SPC)]


ATTN = _attn_chunks()
NMASK = sum(len(c) for c in ATTN)  # 8
MASK_COL = {}
_c = 0
for _s in range(SPC):
    for _ci in range(len(ATTN[_s])):
        MASK_COL[(_s, _ci)] = _c
        _c += 1

VTILES = _chunks(VOUT, VT)    # 58x512 + 305
TCH = _chunks(TOK, 128)       # token chunks: 6x128 + 32


def _build_program(zero_bias):
    nc = bacc.Bacc(None, target_bir_lowering=False, debug=False)
    dt = mybir.dt
    AF = mybir.ActivationFunctionType
    OP = mybir.AluOpType

    from contextlib import ExitStack
    with tile.TileContext(nc) as tc, ExitStack() as ctx:
        dram = ctx.enter_context(tc.tile_pool(name="dram", bufs=1, space="DRAM"))

        def din(name, shape, dtype):
            return dram.tile(shape, dtype, kind="ExternalInput", name=name,
                             uniquify=False)

        idx_d = din("idx", [128, NTCH], dt.int32)
        mask_d = din("mask", [128, NMASK], dt.float32)
        twd = []
        for t in range(4):
            emb, lhs = EMB[t], LHS[t]
            nch1 = len(_chunks(lhs))
            d_ = {
                "table": din(f"t{t}_table", [V, emb], dt.bfloat16),
                "w1T": din(f"t{t}_w1T", [emb, lhs], dt.bfloat16),
                "b1": din(f"t{t}_b1", [128, nch1], dt.float32),
                "w2T": din(f"t{t}_w2T", [nch1 * 128, D], dt.bfloat16),
                "b2": din(f"t{t}_b2", [128, 2], dt.float32),
                "posT": din(f"t{t}_posT", [D, L], dt.float32),
                "ln0": din(f"t{t}_ln0", [128, 4], dt.float32),
                "top": din(f"t{t}_top", [128, 4], dt.float32),
                "layers": [],
            }
            for l in range(NL):
                p = f"t{t}_l{l}_"
                d_["layers"].append({
                    "wqT": din(p + "wqT", [D, D], dt.bfloat16),
                    "wkT": din(p + "wkT", [D, D], dt.bfloat16),
                    "wvT": din(p + "wvT", [D, D], dt.bfloat16),
                    "woT": din(p + "woT", [D, D], dt.bfloat16),
                    "bq": din(p + "bq", [128, 2], dt.float32),
                    "bk": din(p + "bk", [128, 2], dt.float32),
                    "bv": din(p + "bv", [D], dt.float32),
                    "bo": din(p + "bo", [128, 2], dt.float32),
                    "ln1": din(p + "ln1", [128, 4], dt.float32),
                    "ln2": din(p + "ln2", [128, 4], dt.float32),
                    "fw1T": din(p + "fw1T", [D, FFD], dt.bfloat16),
                    "fb1": din(p + "fb1", [128, FFD // 128], dt.float32),
                    "fw2T": din(p + "fw2T", [FFD, D], dt.bfloat16),
                    "fb2": din(p + "fb2", [128, 2], dt.float32),
                })
            twd.append(d_)
        outw_d = din("out_wT", [D, VOUT], dt.bfloat16)
        outb_d = din("out_b", [1, VOUT], dt.bfloat16)
        out_d = dram.tile([TOK, VOUT], dt.float32, kind="ExternalOutput",
                          name="out", uniquify=False)

        const = ctx.enter_context(tc.tile_pool(name="const", bufs=1))
        work = ctx.enter_context(tc.tile_pool(name="work", bufs=2))
        wpool = ctx.enter_context(tc.tile_pool(name="wpool", bufs=2))
        espool = ctx.enter_context(tc.tile_pool(name="espool", bufs=8))
        lpool = ctx.enter_context(tc.tile_pool(name="lpool", bufs=3))
        ps = ctx.enter_context(tc.tile_pool(name="ps", bufs=4, space="PSUM"))

        _PS_BUFS = {"ps": 4, "ps_sum": 2, "ps_rep": 1, "ps_av": 1}

        def psum(p_, f_, tag="ps"):
            return ps.tile([p_, f_], dt.float32, space="PSUM", tag=tag,
                           name=tag, bufs=_PS_BUFS[tag])

        # constants
        ones_stat = const.tile([128, 128], dt.bfloat16)   # value 2^-8
        nc.vector.memset(ones_stat, 2.0 ** -8)
        eps_std = const.tile([128, 1], dt.float32)
        nc.vector.memset(eps_std, 1e-5)
        eps_cus = const.tile([128, 1], dt.float32)
        nc.vector.memset(eps_cus, 1e-6)
        ones_col = const.tile([128, 1], dt.bfloat16)
        nc.vector.memset(ones_col, 1.0)
        ones_row = const.tile([1, 128], dt.bfloat16)
        nc.vector.memset(ones_row, 1.0)
        ident = const.tile([128, 128], dt.bfloat16)
        from concourse.masks import make_identity
        make_identity(nc, ident)
        idx_sb = const.tile([128, NTCH], dt.int32)
        nc.sync.dma_start(idx_sb, idx_d[:])
        mask_sb = const.tile([128, NMASK], dt.float32)
        nc.sync.dma_start(mask_sb, mask_d[:])

        yT = const.tile([128, 2, TOK], dt.float32)   # tower-sum accumulator
        nc.vector.memset(yT[:, 0, :], 0.0)
        nc.vector.memset(yT[:, 1, :], 0.0)

        def layernorm(r, kind, wb_sb, xn):
            """r: [128,2,TOK] bf16 pre-norm. wb_sb: [128,4] (w0,w1,b0,b1).
            Writes xn (bf16 or fp32, per tile dtype)."""
            sq = work.tile([128, 2, TOK], dt.bfloat16, tag="ln_sq", bufs=1)
            for k in range(2):
                nc.vector.tensor_mul(sq[:, k, :], r[:, k, :], r[:, k, :])
            for f in range(2):
                fs = slice(f * FH, (f + 1) * FH)
                pm = psum(128, FH)
                pq = psum(128, FH)
                for k in range(2):
                    nc.tensor.matmul(pm[:], lhsT=ones_stat[:], rhs=r[:, k, fs],
                                     start=(k == 0), stop=(k == 1))
                for k in range(2):
                    nc.tensor.matmul(pq[:], lhsT=ones_stat[:], rhs=sq[:, k, fs],
                                     start=(k == 0), stop=(k == 1))
                # pm = mean (replicated), pq = mean-square (replicated)
                mean_sb = work.tile([128, FH], dt.float32, tag="ln_mean")
                nc.scalar.activation(mean_sb[:], pm[:], AF.Copy)
                var = work.tile([128, FH], dt.float32, tag="ln_var")
                m2 = work.tile([128, FH], dt.float32, tag="ln_m2")
                nc.vector.tensor_mul(m2[:], mean_sb[:], mean_sb[:])
                nc.vector.tensor_sub(var[:], pq[:], m2[:])
                if kind == "std":
                    # rstd = 1/sqrt(var + 1e-5)
                    nc.scalar.activation(var[:], var[:], AF.Abs_reciprocal_sqrt,
                                         bias=eps_std[:])
                else:
                    # 1/(sqrt(var*256/255) + 1e-6) ~= ars(var*256/255)
                    # (eps/s <= 1e-5 relative, far below bf16 noise)
                    nc.scalar.activation(var[:], var[:], AF.Abs_reciprocal_sqrt,
                                         scale=256.0 / 255.0)
                for k in range(2):
                    xs = xn[:, k, fs]
                    nc.vector.tensor_sub(xs, r[:, k, fs], mean_sb[:])
                    nc.vector.tensor_mul(xs, xs, var[:])
                    nc.vector.tensor_scalar(
                        out=xs, in0=xs,
                        scalar1=wb_sb[:, k:k + 1], scalar2=wb_sb[:, 2 + k:3 + k],
                        op0=OP.mult, op1=OP.add)

        for t in range(4):
            emb, lhs = EMB[t], LHS[t]
            nesub = emb // 128
            lch = _chunks(lhs)
            td = twd[t]

            # ---- embedding gather + transpose -> ET [128, nesub, TOKP] bf16
            ET = work.tile([128, nesub, TOKP], dt.bfloat16, tag="ET", bufs=2)
            for k in range(NTCH):
                eg = work.tile([128, emb], dt.bfloat16, tag="eg", bufs=3)
                nc.gpsimd.indirect_dma_start(
                    out=eg[:], out_offset=None, in_=td["table"][:],
                    in_offset=bass.IndirectOffsetOnAxis(
                        ap=idx_sb[:, k:k + 1], axis=0))
                for s in range(nesub):
                    pt = ps.tile([128, 128], dt.bfloat16, space="PSUM",
                                 tag="ps", name="pt", bufs=4)
                    nc.tensor.transpose(pt[:], eg[:, 128 * s:128 * (s + 1)],
                                        ident[:])
                    nc.vector.tensor_copy(ET[:, s, 128 * k:128 * (k + 1)], pt[:])

            # ---- MLP1: A1T = lrelu(w1 @ E.T + b1)
            w1_sb = wpool.tile([128, nesub, lhs], dt.bfloat16, tag="w1", bufs=1)
            nc.sync.dma_start(w1_sb, td["w1T"][:].rearrange(
                "(e p) l -> p e l", p=128))
            b1_sb = wpool.tile([128, len(lch)], dt.float32, tag="b1")
            nc.sync.dma_start(b1_sb, td["b1"][:])
            A1T = work.tile([128, len(lch), TOK], dt.bfloat16, tag="A1T", bufs=1)
            for c, (ms, msz) in enumerate(lch):
                pf = [psum(128, FH), psum(128, FH)]
                for s in range(nesub):
                    for f in range(2):
                        nc.tensor.matmul(pf[f][:msz],
                                         lhsT=w1_sb[:, s, ms:ms + msz],
                                         rhs=ET[:, s, f * FH:(f + 1) * FH],
                                         start=(s == 0), stop=(s == nesub - 1))
                for f in range(2):
                    nc.scalar.activation(A1T[:msz, c, f * FH:(f + 1) * FH],
                                         pf[f][:msz], AF.Lrelu,
                                         bias=b1_sb[:msz, c:c + 1], alpha=0.01)

            # ---- MLP2: h = lrelu(w2 @ A1.T + b2) -> r fp32 [128,2,TOK]
            w2_sb = wpool.tile([128, len(lch), D], dt.bfloat16, tag="w2")
            nc.sync.dma_start(w2_sb, td["w2T"][:].rearrange(
                "(c p) d -> p c d", p=128))
            b2_sb = wpool.tile([128, 2], dt.float32, tag="b2")
            nc.sync.dma_start(b2_sb, td["b2"][:])
            pos_sb = wpool.tile([128, 2, L], dt.float32, tag="pos")
            nc.sync.dma_start(pos_sb, td["posT"][:].rearrange(
                "(k p) t -> p k t", p=128))
            r = work.tile([128, 2, TOK], dt.bfloat16, tag="resid")
            for m in range(2):
                pf = [psum(128, FH), psum(128, FH)]
                for c, (ks, ksz) in enumerate(lch):
                    for f in range(2):
                        nc.tensor.matmul(pf[f][:],
                                         lhsT=w2_sb[:ksz, c,
                                                    128 * m:128 * (m + 1)],
                                         rhs=A1T[:ksz, c, f * FH:(f + 1) * FH],
                                         start=(c == 0), stop=(c == len(lch) - 1))
                for f in range(2):
                    nc.scalar.activation(r[:, m, f * FH:(f + 1) * FH],
                                         pf[f][:], AF.Lrelu,
                                         bias=b2_sb[:, m:m + 1], alpha=0.01)
            # + positional embedding (per sequence)
            for m in range(2):
                for s in range(SPC):
                    sl = slice(s * L, s * L + L)
                    nc.vector.tensor_add(r[:, m, sl], r[:, m, sl],
                                         pos_sb[:, m, :])

            # ---- ln0 (std)
            ln0_sb = wpool.tile([128, 4], dt.float32, tag="ln0")
            nc.sync.dma_start(ln0_sb, td["ln0"][:])
            xn = work.tile([128, 2, TOK], dt.bfloat16, tag="xn")
            layernorm(r, "std", ln0_sb, xn)

            # ---- transformer layers
            for l in range(NL):
                ld = td["layers"][l]
                wq_sb = wpool.tile([128, 2, D], dt.bfloat16, tag="wq")
                wk_sb = wpool.tile([128, 2, D], dt.bfloat16, tag="wk")
                wv_sb = wpool.tile([128, 2, D], dt.bfloat16, tag="wv")
                wo_sb = wpool.tile([128, 2, D], dt.bfloat16, tag="wo")
                for sb_, nm in ((wq_sb, "wqT"), (wk_sb, "wkT"),
                                (wv_sb, "wvT"), (wo_sb, "woT")):
                    nc.sync.dma_start(sb_, ld[nm][:].rearrange(
                        "(k p) d -> p k d", p=128))
                bq_sb = wpool.tile([128, 2], dt.float32, tag="bq")
                nc.sync.dma_start(bq_sb, ld["bq"][:])
                bk_sb = wpool.tile([128, 2], dt.float32, tag="bk")
                nc.sync.dma_start(bk_sb, ld["bk"][:])
                bo_sb = wpool.tile([128, 2], dt.float32, tag="bo")
                nc.sync.dma_start(bo_sb, ld["bo"][:])
                bv_rep = wpool.tile([128, D], dt.float32, tag="bv")
                nc.sync.dma_start(bv_rep, bass.AP(
                    tensor=ld["bv"].tensor, offset=ld["bv"].offset,
                    ap=[[0, 128]] + list(ld["bv"].ap)))

                # Q.T, K.T: [128, 2, TOK] bf16
                QT = work.tile([128, 2, TOK], dt.bfloat16, tag="QT")
                KT = work.tile([128, 2, TOK], dt.bfloat16, tag="KT")
                for dst, w_sb, bias_sb in ((QT, wq_sb, bq_sb), (KT, wk_sb, bk_sb)):
                    for m in range(2):
                        pf = [psum(128, FH), psum(128, FH)]
                        for k in range(2):
                            for f in range(2):
                                nc.tensor.matmul(
                                    pf[f][:],
                                    lhsT=w_sb[:, k, 128 * m:128 * (m + 1)],
                                    rhs=xn[:, k, f * FH:(f + 1) * FH],
                                    start=(k == 0), stop=(k == 1))
                        for f in range(2):
                            nc.vector.tensor_scalar_add(
                                dst[:, m, f * FH:(f + 1) * FH], pf[f][:],
                                bias_sb[:, m:m + 1])
                # V (token-major, per-sequence aligned): [128, 2*SPC, D] bf16
                Vb = work.tile([128, 2 * SPC, D], dt.bfloat16, tag="Vb", bufs=1)
                for s in range(SPC):
                    for c2, (js, jsz) in enumerate(ATTN[s]):
                        pp = psum(128, D)
                        for k in range(2):
                            nc.tensor.matmul(pp[:jsz],
                                             lhsT=xn[:, k, js:js + jsz],
                                             rhs=wv_sb[:, k, :],
                                             start=(k == 0), stop=(k == 1))
                        nc.vector.tensor_add(Vb[:jsz, 2 * s + c2, :], pp[:jsz],
                                             bv_rep[:jsz, :])

                # attention -> OT [128, 2, TOK] bf16
                OT = work.tile([128, 2, TOK], dt.bfloat16, tag="OT")
                for s in range(SPC):
                    sl = slice(s * L, s * L + L)
                    nch = len(ATTN[s])
                    es_all = {}
                    pss = [psum(1, 2 * L, tag="ps_sum") for _ in range(2)]
                    for h in range(H):
                        k2, r0 = h // 2, 64 * (h % 2)
                        rr = slice(r0, r0 + 64)
                        sumsl = slice((h % 2) * L, (h % 2) * L + L)
                        for ci, (gs, sz) in enumerate(ATTN[s]):
                            pS = psum(128, L)
                            nc.tensor.matmul(pS[:sz], lhsT=KT[rr, k2, gs:gs + sz],
                                             rhs=QT[rr, k2, sl],
                                             start=True, stop=True)
                            es = espool.tile([128, L], dt.bfloat16, tag="es")
                            mc = MASK_COL[(s, ci)]
                            nc.scalar.activation(
                                es[:sz], pS[:sz], AF.Exp,
                                bias=mask_sb[:sz, mc:mc + 1],
                                scale=1.0 / math.sqrt(DK))
                            es_all[(h, ci)] = (es, sz)
                            nc.tensor.matmul(pss[h // 2][:, sumsl],
                                             lhsT=ones_col[:sz], rhs=es[:sz],
                                             start=(ci == 0), stop=(ci == nch - 1))
                    # 1/Z = ars(Z)^2: ars on scalar row, replicate, square
                    rep_sb = espool.tile([128, H * L], dt.bfloat16, tag="rep_sb",
                                         bufs=2)
                    for g in range(2):
                        rsum = espool.tile([1, 2 * L], dt.bfloat16, tag="rsum")
                        nc.scalar.activation(rsum[:], pss[g][:],
                                             AF.Abs_reciprocal_sqrt)
                        prep = psum(128, 2 * L, tag="ps_rep")
                        nc.tensor.matmul(prep[:], lhsT=ones_row[:],
                                         rhs=rsum[:], start=True, stop=True)
                        nc.scalar.activation(
                            rep_sb[:, 2 * g * L:2 * (g + 1) * L], prep[:],
                            AF.Square)
                    for h in range(H):
                        k2, r0 = h // 2, 64 * (h % 2)
                        rr = slice(r0, r0 + 64)
                        pO = psum(64, L, tag="ps_av")
                        for ci in range(nch):
                            es, sz = es_all[(h, ci)]
                            nc.tensor.matmul(pO[:], lhsT=Vb[:sz, 2 * s + ci,
                                                            64 * h:64 * h + 64],
                                             rhs=es[:sz],
                                             start=(ci == 0), stop=(ci == nch - 1))
                        nc.vector.tensor_mul(OT[rr, k2, sl], pO[:],
                                             rep_sb[:64, h * L:(h + 1) * L])

                # out-proj + residual -> r_new
                r_new = work.tile([128, 2, TOK], dt.bfloat16, tag="resid")
                for m in range(2):
                    pf = [psum(128, FH), psum(128, FH)]
                    for k in range(2):
                        for f in range(2):
                            nc.tensor.matmul(pf[f][:],
                                             lhsT=wo_sb[:, k, 128 * m:128 * (m + 1)],
                                             rhs=OT[:, k, f * FH:(f + 1) * FH],
                                             start=(k == 0), stop=(k == 1))
                    for f in range(2):
                        fs = slice(f * FH, (f + 1) * FH)
                        nc.vector.scalar_tensor_tensor(
                            out=r_new[:, m, fs], in0=pf[f][:],
                            scalar=bo_sb[:, m:m + 1], in1=xn[:, m, fs],
                            op0=OP.add, op1=OP.add)
                ln1_sb = wpool.tile([128, 4], dt.float32, tag="ln1")
                nc.sync.dma_start(ln1_sb, ld["ln1"][:])
                xn = work.tile([128, 2, TOK], dt.bfloat16, tag="xn")
                layernorm(r_new, "custom", ln1_sb, xn)

                # FF
                fw1_sb = wpool.tile([128, 2, FFD], dt.bfloat16, tag="fw1", bufs=1)
                nc.sync.dma_start(fw1_sb, ld["fw1T"][:].rearrange(
                    "(k p) f -> p k f", p=128))
                fb1_sb = wpool.tile([128, FFD // 128], dt.float32, tag="fb1")
                nc.sync.dma_start(fb1_sb, ld["fb1"][:])
                fw2_sb = wpool.tile([128, FFD // 128, D], dt.bfloat16, tag="fw2", bufs=1)
                nc.sync.dma_start(fw2_sb, ld["fw2T"][:].rearrange(
                    "(c p) d -> p c d", p=128))
                fb2_sb = wpool.tile([128, 2], dt.float32, tag="fb2")
                nc.sync.dma_start(fb2_sb, ld["fb2"][:])

                G = work.tile([128, FFD // 128, TOK], dt.bfloat16, tag="G", bufs=2)
                FQ = TOK // 4
                for c in range(FFD // 128):
                    pf = [psum(128, FQ) for _ in range(4)]
                    for k in range(2):
                        for f in range(4):
                            nc.tensor.matmul(pf[f][:],
                                             lhsT=fw1_sb[:, k, 128 * c:128 * (c + 1)],
                                             rhs=xn[:, k, f * FQ:(f + 1) * FQ],
                                             start=(k == 0), stop=(k == 1))
                    for f in range(4):
                        nc.scalar.activation(G[:, c, f * FQ:(f + 1) * FQ],
                                             pf[f][:], AF.Gelu_apprx_tanh,
                                             bias=fb1_sb[:, c:c + 1])
                r_new = work.tile([128, 2, TOK], dt.bfloat16, tag="resid")
                for m in range(2):
                    pf = [psum(128, FQ) for _ in range(4)]
                    for c in range(FFD // 128):
                        for f in range(4):
                            nc.tensor.matmul(pf[f][:],
                                             lhsT=fw2_sb[:, c, 128 * m:128 * (m + 1)],
                                             rhs=G[:, c, f * FQ:(f + 1) * FQ],
                                             start=(c == 0),
                                             stop=(c == FFD // 128 - 1))
                    for f in range(4):
                        fs = slice(f * FQ, (f + 1) * FQ)
                        nc.vector.scalar_tensor_tensor(
                            out=r_new[:, m, fs], in0=pf[f][:],
                            scalar=fb2_sb[:, m:m + 1], in1=xn[:, m, fs],
                            op0=OP.add, op1=OP.add)
                ln2_sb = wpool.tile([128, 4], dt.float32, tag="ln2")
                nc.sync.dma_start(ln2_sb, ld["ln2"][:])
                xn = work.tile([128, 2, TOK], dt.bfloat16, tag="xn")
                layernorm(r_new, "custom", ln2_sb, xn)

            # ---- top std_ln, accumulate into yT
            top_sb = wpool.tile([128, 4], dt.float32, tag="top")
            nc.sync.dma_start(top_sb, td["top"][:])
            tn = work.tile([128, 2, TOK], dt.float32, tag="tn")
            layernorm(xn, "std", top_sb, tn)
            for k in range(2):
                nc.vector.tensor_add(yT[:, k, :], yT[:, k, :], tn[:, k, :])

        # ---- final projection: out[tok, vocab] = y @ out_w.T + out_b
        yTb = const.tile([128, 2, TOK], dt.bfloat16)
        for k in range(2):
            nc.vector.tensor_copy(yTb[:, k, :], yT[:, k, :])
        ROT = ["ps", "ps", "ps", "ps", "ps_sum", "ps_sum", "ps_rep",
               "ps_av"]
        rot_i = 0
        for ni, (ns, nsz) in enumerate(VTILES):
            w_sb = lpool.tile([128, 2, VT], dt.bfloat16, tag="wv", bufs=3,
                              name="wv")
            nc.sync.dma_start(
                w_sb[:, :, :nsz],
                outw_d[:, ns:ns + nsz].rearrange("(k p) v -> p k v", p=128))
            if not zero_bias:
                ob = lpool.tile([1, VT], dt.bfloat16, tag="ob", bufs=3,
                                name="ob")
                nc.sync.dma_start(ob[:, :nsz], outb_d[0:1, ns:ns + nsz])
                pb = psum(128, VT, tag="ps_rep")
                nc.tensor.matmul(pb[:, :nsz], lhsT=ones_row[:],
                                 rhs=ob[:, :nsz], start=True, stop=True)
                bias_sb = lpool.tile([128, VT], dt.float32, tag="pbias",
                                     bufs=2, name="pbias")
                nc.vector.tensor_copy(bias_sb[:, :nsz], pb[:, :nsz])
            lsb = lpool.tile([128, NTCH, VT], dt.float32, tag="lsb", bufs=2,
                             name="lsb")
            for mi, (ms, msz) in enumerate(TCH):
                pl = psum(128, VT, tag=ROT[rot_i % len(ROT)])
                rot_i += 1
                for k in range(2):
                    nc.tensor.matmul(pl[:msz, :nsz],
                                     lhsT=yTb[:, k, ms:ms + msz],
                                     rhs=w_sb[:, k, :nsz],
                                     start=(k == 0), stop=(k == 1))
                if zero_bias:
                    nc.vector.tensor_copy(lsb[:msz, mi, :nsz], pl[:msz, :nsz])
                else:
                    nc.vector.tensor_add(lsb[:msz, mi, :nsz], pl[:msz, :nsz],
                                         bias_sb[:msz, :nsz])
            nc.gpsimd.dma_start(
                out_d[:768, ns:ns + nsz].rearrange("(c p) v -> p c v", p=128),
                lsb[:, :6, :nsz])
            nc.gpsimd.dma_start(out_d[768:, ns:ns + nsz], lsb[:32, 6, :nsz])

    nc.compile()
    return nc


_NC = {}


def _get_nc(zero_bias):
    if zero_bias not in _NC:
        _NC[zero_bias] = _build_program(zero_bias)
    return _NC[zero_bias]


def _pack_pp(v):
    """[256] -> [128, 2] column-per-dsub layout."""
    return np.ascontiguousarray(v.reshape(2, 128).T.astype(F32))


def _pack_pp_n(v, n=128):
    k = (len(v) + n - 1) // n
    buf = np.zeros((k * n,), F32)
    buf[:len(v)] = v
    return np.ascontiguousarray(buf.reshape(k, n).T)


def _pack_ln(w, b):
    return np.ascontiguousarray(
        np.stack([w[:128], w[128:], b[:128], b[128:]], axis=1).astype(F32))


def _prep_weights(t1, t2, t3, t4, out_w, out_b):
    m = {}
    for t, td in enumerate((t1, t2, t3, t4)):
        lhs = LHS[t]
        nch1 = len(_chunks(lhs))
        m[f"t{t}_table"] = np.asarray(td["table"]).astype(BF16)
        m[f"t{t}_w1T"] = np.ascontiguousarray(
            np.asarray(td["w1"]).T).astype(BF16)
        m[f"t{t}_b1"] = _pack_pp_n(np.asarray(td["b1"]))
        w2T = np.zeros((nch1 * 128, D), F32)
        w2T[:lhs] = np.asarray(td["w2"]).T
        m[f"t{t}_w2T"] = w2T.astype(BF16)
        m[f"t{t}_b2"] = _pack_pp(np.asarray(td["b2"]))
        m[f"t{t}_posT"] = np.ascontiguousarray(
            np.asarray(td["pos"]).T.astype(F32))
        m[f"t{t}_ln0"] = _pack_ln(np.asarray(td["ln0_w"]),
                                  np.asarray(td["ln0_b"]))
        m[f"t{t}_top"] = _pack_ln(np.asarray(td["top_w"]),
                                  np.asarray(td["top_b"]))
        for l, ld in enumerate(td["layers"]):
            p = f"t{t}_l{l}_"
            for nm in ("wq", "wk", "wv", "wo"):
                m[p + nm + "T"] = np.ascontiguousarray(
                    np.asarray(ld[nm]).T).astype(BF16)
            m[p + "bq"] = _pack_pp(np.asarray(ld["bq"]))
            m[p + "bk"] = _pack_pp(np.asarray(ld["bk"]))
            m[p + "bv"] = np.asarray(ld["bv"]).astype(F32)
            m[p + "bo"] = _pack_pp(np.asarray(ld["bo"]))
            m[p + "ln1"] = _pack_ln(np.asarray(ld["a1"]), np.asarray(ld["c1"]))
            m[p + "ln2"] = _pack_ln(np.asarray(ld["a2"]), np.asarray(ld["c2"]))
            m[p + "fw1T"] = np.ascontiguousarray(
                np.asarray(ld["fw1"]).T).astype(BF16)
            m[p + "fb1"] = _pack_pp_n(np.asarray(ld["fb1"]))
            m[p + "fw2T"] = np.ascontiguousarray(
                np.asarray(ld["fw2"]).T).astype(BF16)
            m[p + "fb2"] = _pack_pp(np.asarray(ld["fb2"]))
    m["out_wT"] = np.ascontiguousarray(np.asarray(out_w).T).astype(BF16)
    m["out_b"] = np.asarray(out_b).reshape(1, VOUT).astype(BF16)
    return m


def _prep_core(x, c):
    """Per-core idx [128, NTCH] int32 and mask [128, NMASK] fp32."""
    xs = np.asarray(x[SPC * c:SPC * (c + 1)]).astype(np.int64)  # [4, 200]
    flat = xs.reshape(-1)  # [800]
    idx = np.zeros((NTCH * 128,), np.int32)
    idx[:TOK] = flat.astype(np.int32)
    idx = np.ascontiguousarray(idx.reshape(NTCH, 128).T)
    mask = np.full((128, NMASK), NEG, F32)
    for s in range(SPC):
        for ci, (gs, sz) in enumerate(ATTN[s]):
            col = MASK_COL[(s, ci)]
            valid = xs[s, gs - s * L: gs - s * L + sz] > 0
            mask[:sz, col] = np.where(valid, 0.0, NEG)
    return idx, mask


def _run(inputs, trace=False):
    x = np.asarray(inputs["x"])
    wm = _prep_weights(inputs["t1"], inputs["t2"], inputs["t3"], inputs["t4"],
                       inputs["out_w"], inputs["out_b"])
    zero_bias = not np.any(np.asarray(inputs["out_b"]))
    nc = _get_nc(zero_bias)
    in_maps = []
    for c in range(NCORE):
        idx, mask = _prep_core(x, c)
        m = dict(wm)
        m["idx"] = idx
        m["mask"] = mask
        in_maps.append(m)
    res = run_bass_kernel_spmd(nc, in_maps, core_ids=list(range(NCORE)),
                               trace=trace)
    outs = [res.results[c]["out"].reshape(SPC, L, VOUT) for c in range(NCORE)]
    full = np.concatenate(outs, axis=0).astype(F32)
    return full, res


def kernel(**inputs):
    out, _ = _run(inputs, trace=False)
    return out
